# revision 2
# baseline (speedup 1.0000x reference)
"""CCPL contrastive loss kernel v2 for Trainium2 (8 NeuronCores, SPMD over batch).

Contract: kernel(**inputs) takes FULL unsharded inputs, returns FULL scalar loss.

v2 strategy (vs v1 baseline at ~64us):
  - fp8e4 x/weight blobs (w scaled x16; normalization self-corrects scale),
    DoubleRow matmuls for layer3/layer2 mm1+mm2 (K>=256).
  - Z = sum_t exp(G/tau) via ACT accum_out (no DVE reduces).
  - pos-logit sum via one fused DVE tensor_tensor_reduce per f-group.
  - y2 via one DVE STT ((mm2*(1/16))*mm2) straight from PSUM; the scale
    correction is folded into the rn exp bias (+0.5*ln(1/16)).
  - f via one DVE TT (mm2 * bc) from PSUM (biases are zero per spec fill).
  - relu on DVE tensor_scalar_max pair-ops.
  - layers 0..2 mm2 stacked into one banded PSUM tile (bases 0/32/64, l0
    weight cols zero-padded to 32) -> one shared norm chain for 3 layers.
  - x-subs split: l3-q on DVE (critical), rest on GpSimd.
  - PSUM: P(mm1/G pairs, 2x2 banks) + B(mm2 pair, 2) + C(ssq/bc/final, 2) = 8.
"""

import numpy as np
from contextlib import ExitStack

import ml_dtypes

import concourse.bass as bass
import concourse.bacc as bacc
import concourse.tile as tile
from concourse import mybir
from concourse.bass_utils import run_bass_kernel_spmd

F32 = mybir.dt.float32
F16 = mybir.dt.float16
BF16 = mybir.dt.bfloat16
F8 = mybir.dt.float8e4
NPF8 = ml_dtypes.float8_e4m3

# single ACT table set containing Exp/Ln/Relu/Square
_COMBINED_SET = "natural_log_exp_and_others"
_orig_get_tables = bacc.get_activation_tables


def _patched_get_tables(arch):
    t = _orig_get_tables(arch)
    strip = {
        mybir.ActivationFunctionType.Exp,
        mybir.ActivationFunctionType.Ln,
        mybir.ActivationFunctionType.Relu,
        mybir.ActivationFunctionType.Square,
    }
    return {
        name: (fns if name == _COMBINED_SET else (set(fns) - strip))
        for name, fns in t.items()
    }


bacc.get_activation_tables = _patched_get_tables

TAU = 0.07
NCORES = 8
S = 512
CS = [64, 128, 256, 512]
COUT = [16, 32, 64, 128]
KC = [1, 1, 2, 4]
WSC = 16.0                       # host weight scale (both mm1+mm2)
Y2SC = 1.0 / 16.0                # y2 = (mm2*Y2SC)*mm2 to keep fp16 in range
RN_BIAS = float(0.5 * np.log(Y2SC))
_DH = np.array([0, 0, 0, 1, 1, 2, 2, 2], dtype=np.int64)
_DW = np.array([0, 1, 2, 0, 2, 0, 1, 2], dtype=np.int64)

# x blob chunk order: l3 k0..k3, l2 k0..k1, l1, l0
CHUNK = {(3, 0): 0, (3, 1): 1, (3, 2): 2, (3, 3): 3,
         (2, 0): 4, (2, 1): 5, (1, 0): 6, (0, 0): 7}
NCH = 8
XW = NCH * 576                   # x blob cols (p-major)

# weight blob column offsets (fp8, p-major [128, WTOT])
W1_3 = {(g, m): (g * 4 + m) * 256 for g in range(2) for m in range(4)}
W2_3 = {g: 2048 + g * 256 for g in range(2)}
W1_2 = {m: 2560 + m * 256 for m in range(2)}
W2_2 = 3072          # [128,2,64] -> 128 cols
W1_1 = 3200          # [128,128]
W2_1 = 3328          # [128,32]
W1_0 = 3360          # [64,64]
W2_0 = 3424          # [64,32] (cols 16:32 zero-padded)
WTOT = 3456
WL3_END = 2560       # end of layer-3 weight block

# aux f32 [128, 34]: ones col 12, rn-bias col 13, wvec row 0 cols 14:32
AUXW = 34
ONESF = 12
RNBC = 13
WVC = 14
# auxh fp16 [128, 264]: sel3 lhsT rows0:3 cols0:128; selband [128,3] cols
# 128:131; ones col 131; ones row0 cols 132:260
SEL3C = 0
SELBC = 128
ONESC = 131
ONESR = 132
AUXHW = 264
# stack bands: l0 rows 0:32 (16 real), l1 32:64, l2 64:128
BANDS = {0: (0, 32), 1: (32, 64), 2: (64, 128)}


def _build_nc(use_dr=True, f8x=True, f8w=True):
    nc = bacc.Bacc()
    DTX = F8 if f8x else F16
    DTW = F8 if f8w else F16
    DTH = F8 if f8w else F16
    xq = nc.dram_tensor("xq", [128, XW], DTX, kind="ExternalInput")
    xk = nc.dram_tensor("xk", [128, XW], DTX, kind="ExternalInput")
    wts = nc.dram_tensor("wts", [128, WTOT], DTW, kind="ExternalInput")
    aux = nc.dram_tensor("aux", [128, AUXW], F32, kind="ExternalInput")
    auxh = nc.dram_tensor("auxh", [128, AUXHW], F16, kind="ExternalInput")
    out = nc.dram_tensor("out", [1, 1], F32, kind="ExternalOutput")

    DR = mybir.MatmulPerfMode.DoubleRow
    AL = mybir.AluOpType
    ACT = mybir.ActivationFunctionType

    with ExitStack() as ctx:
        tc = ctx.enter_context(tile.TileContext(nc))
        const = ctx.enter_context(tc.tile_pool(name="const", bufs=1))
        hpool = ctx.enter_context(tc.tile_pool(name="hpool", bufs=1))
        fpool = ctx.enter_context(tc.tile_pool(name="fpool", bufs=1))
        spool = ctx.enter_context(tc.tile_pool(name="spool", bufs=2))
        P = ctx.enter_context(tc.tile_pool(name="P", bufs=2, space="PSUM"))
        B = ctx.enter_context(tc.tile_pool(name="B", bufs=1, space="PSUM"))
        C = ctx.enter_context(tc.tile_pool(name="C", bufs=1, space="PSUM"))

        xq_s = const.tile([128, NCH, 576], DTX)
        xk_s = const.tile([128, NCH, 576], DTX)
        wall = const.tile([128, WTOT], DTW)
        aall = const.tile([128, AUXW], F32)
        hall = const.tile([128, AUXHW], F16)
        ZD = const.tile([128, 16], F32)
        L = const.tile([128, 18], F32)
        res = const.tile([1, 1], F32)
        xsq = const.tile([128, NCH, S], DTX)
        xsk = const.tile([128, NCH, S], DTX)

        rxq = xq.rearrange("p (n m) -> p n m", m=576)
        rxk = xk.rearrange("p (n m) -> p n m", m=576)
        # DMA: small-layer blocks first (schedule is small-layers-first);
        # xq+wts on sync, xk on scalar, aux on gpsimd
        nc.sync.dma_start(out=xq_s[:, 4:8, :], in_=rxq[:, 4:8, :])
        nc.scalar.dma_start(out=xk_s[:, 4:8, :], in_=rxk[:, 4:8, :])
        nc.sync.dma_start(out=wall[:, WL3_END:WTOT], in_=wts[:, WL3_END:WTOT])
        nc.gpsimd.dma_start(out=hall, in_=auxh[:, :])
        nc.sync.dma_start(out=xq_s[:, 0:4, :], in_=rxq[:, 0:4, :])
        nc.scalar.dma_start(out=xk_s[:, 0:4, :], in_=rxk[:, 0:4, :])
        nc.sync.dma_start(out=wall[:, 0:WL3_END], in_=wts[:, 0:WL3_END])
        nc.gpsimd.dma_start(out=aall, in_=aux[:, :])

        def sub(eng, xall, xs, c0, c1):
            in0 = xall[:, c0:c1, 0:512].rearrange("p n (s j) -> p n s j", j=8)
            cb = xall[:, c0:c1, 512:576]
            in1 = bass.AP(cb.tensor, cb.offset, [*cb.ap, [0, 8]])
            eng.tensor_sub(
                out=xs[:, c0:c1, :].rearrange("p n (s j) -> p n s j", j=8),
                in0=in0, in1=in1)

        # x-subs: small-q on DVE (lead-in); small-k then l3 on GpSimd
        sub(nc.vector, xq_s, xsq, 4, 6)
        sub(nc.vector, xq_s, xsq, 6, 8)
        sub(nc.gpsimd, xk_s, xsk, 4, 6)
        sub(nc.gpsimd, xk_s, xsk, 6, 8)
        sub(nc.gpsimd, xq_s, xsq, 0, 2)
        sub(nc.gpsimd, xq_s, xsq, 2, 4)
        sub(nc.gpsimd, xk_s, xsk, 0, 2)
        sub(nc.gpsimd, xk_s, xsk, 2, 4)

        def wdr(c0, m):
            # DR lhsT block [128, 2, m] at col c0
            return wall[:, c0:c0 + 2 * m].rearrange("p (j m) -> p j m", j=2)

        def relu(out_ap, in_ap):
            nc.vector.tensor_scalar_max(out=out_ap, in0=in_ap, scalar1=0.0)

        def mlp3_mm1_pair(xs, bi, half):
            pm = P.tile([128, 2, S], F32, tag="pg")
            for sl in range(2):
                m = half * 2 + sl
                if use_dr:
                    for g in range(2):
                        nc.tensor.matmul(
                            pm[:, sl, :], lhsT=wdr(W1_3[(g, m)], 128),
                            rhs=xs[:, 2 * g:2 * g + 2, :],
                            start=(g == 0), stop=(g == 1), perf_mode=DR)
                else:
                    for g in range(2):
                        for j in range(2):
                            c0 = W1_3[(g, m)] + 128 * j
                            nc.tensor.matmul(
                                pm[:, sl, :], lhsT=wall[:, c0:c0 + 128],
                                rhs=xs[:, 2 * g + j, :],
                                start=(g == 0 and j == 0),
                                stop=(g == 1 and j == 1))
            return pm

        def mlp3_relu(pm, bi, half):
            h = hpool.tile([128, 2, S], DTH, tag=f"h3{bi}{half}")
            relu(h[:, :, :], pm[:, :, :])
            return h

        def mm2_3(hts, bi, mm2p):
            if use_dr:
                for g in range(2):
                    nc.tensor.matmul(
                        mm2p[:, bi, :], lhsT=wdr(W2_3[g], 128),
                        rhs=hts[g][:, :, :],
                        start=(g == 0), stop=(g == 1), perf_mode=DR)
            else:
                for g in range(2):
                    for j in range(2):
                        c0 = W2_3[g] + 128 * j
                        nc.tensor.matmul(
                            mm2p[:, bi, :], lhsT=wall[:, c0:c0 + 128],
                            rhs=hts[g][:, j, :],
                            start=(g == 0 and j == 0),
                            stop=(g == 1 and j == 1))

        def mlp2_mm1(xs, bi):
            pm = P.tile([128, 2, S], F32, tag="pg")
            for m in range(2):
                if use_dr:
                    nc.tensor.matmul(
                        pm[:, m, :], lhsT=wdr(W1_2[m], 128), rhs=xs[:, 4:6, :],
                        start=True, stop=True, perf_mode=DR)
                else:
                    for j in range(2):
                        c0 = W1_2[m] + 128 * j
                        nc.tensor.matmul(
                            pm[:, m, :], lhsT=wall[:, c0:c0 + 128],
                            rhs=xs[:, 4 + j, :],
                            start=(j == 0), stop=(j == 1))
            h = hpool.tile([128, 2, S], DTH, tag=f"h2{bi}")
            relu(h[:, :, :], pm[:, :, :])
            return h

        def mlp01_mm1(xs, bi):
            pm = P.tile([128, 2, S], F32, tag="pg")
            nc.tensor.matmul(pm[:, 0, :], lhsT=wall[:, W1_1:W1_1 + 128],
                             rhs=xs[:, 6, :], start=True, stop=True)
            nc.tensor.matmul(pm[0:64, 1, :], lhsT=wall[0:64, W1_0:W1_0 + 64],
                             rhs=xs[0:64, 7, :], start=True, stop=True)
            h = hpool.tile([128, 2, S], F16, tag=f"h01{bi}")
            relu(h[:, 0, :], pm[:, 0, :])
            relu(h[0:64, 1, :], pm[0:64, 1, :])
            return h

        def mm2_small(h2, h01, bi, mm2p):
            # l2 mm2: 2 normal fp8 MMs (DR with 64-col tiles fails ISA check)
            for j in range(2):
                nc.tensor.matmul(mm2p[64:128, bi, :],
                                 lhsT=wall[:, W2_2 + 64 * j:W2_2 + 64 * (j + 1)],
                                 rhs=h2[:, j, :], start=(j == 0), stop=(j == 1))
            nc.tensor.matmul(mm2p[32:64, bi, :],
                             lhsT=wall[:, W2_1:W2_1 + 32],
                             rhs=h01[:, 0, :], start=True, stop=True)
            nc.tensor.matmul(mm2p[0:32, bi, :],
                             lhsT=wall[0:64, W2_0:W2_0 + 32],
                             rhs=h01[0:64, 1, :], start=True, stop=True)

        def norm_yp_tile(tagp):
            return spool.tile([128, 2, S], F16, tag=f"yp{tagp}", name=f"yp{tagp}")

        def norm_yp_b(yp, mm2p, bi):
            nc.vector.tensor_scalar_mul(out=yp[:, bi, :], in0=mm2p[:, bi, :],
                                        scalar1=0.25)

        def norm_y2_tile(tagp):
            return spool.tile([128, 2, S], F16, tag=f"y2{tagp}", name=f"y2{tagp}")

        def norm_y2_b(eng, y2, yp, bi):
            eng.tensor_mul(out=y2[:, bi, :], in0=yp[:, bi, :],
                           in1=yp[:, bi, :])

        def norm_ssq_tile():
            return C.tile([128, 2, S], F32, tag="c", name="ssqt")

        def norm_ssq_b(ssq_t, y2, nb, bi):
            lhsT = (hall[0:128, ONESC:ONESC + 1] if nb == 1
                    else hall[:, SELBC:SELBC + 3])
            nc.tensor.matmul(ssq_t[0:nb, bi, :], lhsT=lhsT,
                             rhs=y2[:, bi, :], start=True, stop=True)

        def norm_lnexp(ssq_t, nb, tagp):
            t1 = spool.tile([nb, 2, S], F32, tag=f"t1{tagp}")
            nc.scalar.activation(out=t1[:, :, :], in_=ssq_t[0:nb, :, :],
                                 func=ACT.Ln)
            rn = spool.tile([nb, 2, S], F16, tag=f"rn{tagp}")
            nc.scalar.activation(out=rn[:, :, :], in_=t1[:, :, :],
                                 func=ACT.Exp, scale=-0.5)
            return rn

        def norm_bc(rn, nb):
            bc = C.tile([128, 2, S], F32, tag="c")
            blhsT = (hall[0:1, ONESR:ONESR + 128] if nb == 1
                     else hall[0:3, SEL3C:SEL3C + 128])
            for bi in range(2):
                nc.tensor.matmul(bc[:, bi, :], lhsT=blhsT, rhs=rn[:, bi, :],
                                 start=True, stop=True)
            return bc

        def norm_f(yp, bc, tagp):
            f = fpool.tile([128, 2, S], F16, tag=f"f{tagp}")
            nc.vector.tensor_mul(out=f[:, :, :], in0=yp[:, :, :],
                                 in1=bc[:, :, :])
            return f

        def pos(f, col):
            pp = spool.tile([128, S], F16, tag="pp")
            nc.vector.scalar_tensor_tensor(
                out=pp[:, :], in0=f[:, 0, :], scalar=0.0, in1=f[:, 1, :],
                op0=AL.bypass, op1=AL.mult,
                accum_out=L[:, col:col + 1])

        def gram_pair(f, p0, p1, zcol, m0, acc=True):
            g = P.tile([128, 2, S], F32, tag="pg")
            for sl in range(2):
                m = m0 + sl
                nc.tensor.matmul(
                    g[:, sl, :], lhsT=f[p0:p1, 0, m * 128:(m + 1) * 128],
                    rhs=f[p0:p1, 1, :], start=True, stop=True)
            for sl in range(2):
                E = spool.tile([128, S], BF16, tag="E")
                zc = zcol + m0 + sl
                if acc:
                    nc.scalar.activation(
                        out=E[:, :], in_=g[:, sl, :], func=ACT.Exp,
                        scale=1.0 / TAU, accum_out=ZD[:, zc:zc + 1])
                else:
                    nc.scalar.activation(
                        out=E[:, :], in_=g[:, sl, :], func=ACT.Exp,
                        scale=1.0 / TAU)
                    nc.vector.reduce_sum(out=ZD[:, zc:zc + 1], in_=E[:, :],
                                         axis=mybir.AxisListType.X)

        # ---- schedule: small layers lead, per-branch norm, l3 fills ----
        mm2ps = B.tile([128, 2, S], F32, tag="mm2")
        h2q = mlp2_mm1(xsq, 0)
        h01q = mlp01_mm1(xsq, 0)
        mm2_small(h2q, h01q, 0, mm2ps)
        yps = norm_yp_tile("s")
        y2s = norm_y2_tile("s")
        ssqs = norm_ssq_tile()
        norm_yp_b(yps, mm2ps, 0)
        norm_y2_b(nc.vector, y2s, yps, 0)
        norm_ssq_b(ssqs, y2s, 3, 0)
        h2k = mlp2_mm1(xsk, 1)
        h01k = mlp01_mm1(xsk, 1)
        mm2_small(h2k, h01k, 1, mm2ps)
        pm3qA = mlp3_mm1_pair(xsq, 0, 0)
        norm_yp_b(yps, mm2ps, 1)
        norm_y2_b(nc.vector, y2s, yps, 1)
        norm_ssq_b(ssqs, y2s, 3, 1)
        rns = norm_lnexp(ssqs, 3, "s")
        h3qA = mlp3_relu(pm3qA, 0, 0)
        pm3qB = mlp3_mm1_pair(xsq, 0, 1)
        bcs = norm_bc(rns, 3)
        fs = norm_f(yps, bcs, "s")
        pos(fs, 16)
        gram_pair(fs, 64, 128, 4, 0)    # l2
        h3qB = mlp3_relu(pm3qB, 0, 1)
        pm3kA = mlp3_mm1_pair(xsk, 1, 0)
        gram_pair(fs, 64, 128, 4, 2)
        mm2p3 = B.tile([128, 2, S], F32, tag="mm2")
        mm2_3([h3qA, h3qB], 0, mm2p3)
        gram_pair(fs, 32, 64, 8, 0)     # l1
        h3kA = mlp3_relu(pm3kA, 1, 0)
        pm3kB = mlp3_mm1_pair(xsk, 1, 1)
        h3kB = mlp3_relu(pm3kB, 1, 1)
        gram_pair(fs, 32, 64, 8, 2)     # l1
        mm2_3([h3kA, h3kB], 1, mm2p3)
        yp3 = norm_yp_tile("3")
        norm_yp_b(yp3, mm2p3, 0)
        norm_yp_b(yp3, mm2p3, 1)
        y23 = norm_y2_tile("3")
        norm_y2_b(nc.vector, y23, yp3, 0)
        norm_y2_b(nc.vector, y23, yp3, 1)
        ssq3 = norm_ssq_tile()
        norm_ssq_b(ssq3, y23, 1, 0)
        norm_ssq_b(ssq3, y23, 1, 1)
        rn3 = norm_lnexp(ssq3, 1, "3")
        gram_pair(fs, 0, 16, 12, 0, acc=False)     # l0
        bc3 = norm_bc(rn3, 1)
        f3 = norm_f(yp3, bc3, "3")
        pos(f3, 17)
        gram_pair(fs, 0, 16, 12, 2, acc=False)     # l0
        gram_pair(f3, 0, 128, 0, 0, acc=False)     # l3
        gram_pair(f3, 0, 128, 0, 2, acc=False)
        # final combine
        nc.scalar.activation(out=L[:, 0:16], in_=ZD[:, :], func=ACT.Ln)
        fin_t = C.tile([128, 2, S], F32, tag="c")
        fin = fin_t[0:1, 0, 0:18]
        nc.tensor.matmul(fin, lhsT=aall[:, ONESF:ONESF + 1],
                         rhs=L[:, :], start=True, stop=True)
        wp = spool.tile([1, 18], F32, tag="wp")
        nc.vector.scalar_tensor_tensor(
            out=wp[:, :], in0=fin, scalar=0.0, in1=aall[0:1, WVC:WVC + 18],
            op0=AL.bypass, op1=AL.mult, accum_out=res[:, :])
        nc.sync.dma_start(out=out[:, :], in_=res[:, :])
    nc.finalize()
    return nc


_NC_CACHE = {}


import os
def _flags():
    use_dr = os.environ.get("V2_DR", "1") == "1"
    f8x = os.environ.get("V2_F8X", "1") == "1"
    f8w = os.environ.get("V2_F8W", "1") == "1"
    if not f8w:
        use_dr = False
    return use_dr, f8x, f8w


def _get_nc():
    key = _flags()
    if key not in _NC_CACHE:
        _NC_CACHE[key] = _build_nc(*key)
    return _NC_CACHE[key]


def _to_f8(a):
    _, f8x, f8w = _flags()
    return np.clip(a, -224.0, 224.0).astype(NPF8)


def _to_x(a):
    return np.clip(a, -224.0, 224.0).astype(
        NPF8 if _flags()[1] else np.float16)


def _to_w(a):
    return np.clip(a, -224.0, 224.0).astype(
        NPF8 if _flags()[2] else np.float16)


def _host_blobs(inputs):
    nidx, cidx = [], []
    for l in range(4):
        sid = np.asarray(inputs[f"sid{l}"]).astype(np.int64)
        nidx.append(((sid[:, 0:1] + _DH) * 32 + (sid[:, 1:2] + _DW)).reshape(-1))
        cidx.append((sid[:, 0] + 1) * 32 + (sid[:, 1] + 1))

    np_w = NPF8 if _flags()[2] else np.float16
    np_x = NPF8 if _flags()[1] else np.float16
    wts = np.zeros((128, WTOT), dtype=np_w)

    def putw(col, blk):
        w = blk.shape[1] * blk.shape[2]
        wts[:blk.shape[0], col:col + w] = _to_w(blk.reshape(blk.shape[0], -1))

    w1 = {l: np.asarray(inputs[f"w1_{l}"]).astype(np.float64) * WSC
          for l in range(4)}
    w2 = {l: np.asarray(inputs[f"w2_{l}"]).astype(np.float64) * WSC
          for l in range(4)}
    w1T, w2T = w1[3].T, w2[3].T
    for g in range(2):
        for m in range(4):
            blk = np.stack([w1T[g * 256 + j * 128:g * 256 + (j + 1) * 128,
                                m * 128:(m + 1) * 128] for j in range(2)], 1)
            putw(W1_3[(g, m)], blk)
        blk = np.stack([w2T[g * 256 + j * 128:g * 256 + (j + 1) * 128, :]
                        for j in range(2)], 1)
        putw(W2_3[g], blk)
    w1T, w2T = w1[2].T, w2[2].T
    for m in range(2):
        blk = np.stack([w1T[j * 128:(j + 1) * 128, m * 128:(m + 1) * 128]
                        for j in range(2)], 1)
        putw(W1_2[m], blk)
    putw(W2_2, np.stack([w2T[j * 128:(j + 1) * 128, :] for j in range(2)], 1))
    wts[:, W1_1:W1_1 + 128] = _to_w(w1[1].T)
    wts[:, W2_1:W2_1 + 32] = _to_w(w2[1].T)
    wts[0:64, W1_0:W1_0 + 64] = _to_w(w1[0].T)
    wts[0:64, W2_0:W2_0 + 16] = _to_w(w2[0].T)

    aux = np.zeros((128, AUXW), dtype=np.float32)
    aux[:, ONESF] = 1.0
    aux[:, RNBC] = RN_BIAS
    aux[0, WVC:WVC + 16] = 1.0
    aux[0, WVC + 16:WVC + 18] = -1.0 / TAU

    auxh = np.zeros((128, AUXHW), dtype=np.float16)
    for l in range(3):
        lo, hi = BANDS[l]
        auxh[l, SEL3C + lo:SEL3C + hi] = 1.0
        auxh[lo:hi, SELBC + l] = 1.0
    auxh[:, ONESC] = 1.0
    auxh[0, ONESR:ONESR + 128] = 1.0

    xqs = [np.zeros((128, XW), dtype=np_x) for _ in range(NCORES)]
    xks = [np.zeros((128, XW), dtype=np_x) for _ in range(NCORES)]
    for l in range(4):
        C = CS[l]
        fq = np.asarray(inputs[f"fq{l}"])[:, :, :32, :32].reshape(NCORES, C, 1024)
        fk = np.asarray(inputs[f"fk{l}"])[:, :, :32, :32].reshape(NCORES, C, 1024)
        qn = _to_x(fq[:, :, nidx[l]])
        qc = _to_x(fq[:, :, cidx[l]])
        kn = _to_x(fk[:, :, nidx[l]])
        kc_ = _to_x(fk[:, :, cidx[l]])
        for kk in range(KC[l]):
            c = CHUNK[(l, kk)]
            rows = min(128, C - kk * 128)
            sl = slice(kk * 128, kk * 128 + rows)
            for b in range(NCORES):
                xqs[b][0:rows, c * 576:c * 576 + 512] = qn[b, sl, :]
                xqs[b][0:rows, c * 576 + 512:c * 576 + 576] = qc[b, sl, :]
                xks[b][0:rows, c * 576:c * 576 + 512] = kn[b, sl, :]
                xks[b][0:rows, c * 576 + 512:c * 576 + 576] = kc_[b, sl, :]
    return wts, aux, auxh, xqs, xks


_LAST_RESULT = {}


def kernel(**inputs):
    assert int(inputs.get("start_layer", 0)) == 0
    assert int(inputs.get("end_layer", 4)) == 4
    assert int(inputs.get("num_s", 64)) == 64
    for l in range(4):
        assert not np.any(np.asarray(inputs[f"b1_{l}"]))
        assert not np.any(np.asarray(inputs[f"b2_{l}"]))

    nc = _get_nc()
    wts, aux, auxh, xqs, xks = _host_blobs(inputs)
    in_maps = [
        {"xq": xqs[b], "xk": xks[b], "wts": wts, "aux": aux, "auxh": auxh}
        for b in range(NCORES)
    ]
    r = run_bass_kernel_spmd(nc, in_maps, core_ids=list(range(NCORES)))
    _LAST_RESULT["r"] = r
    partials = [np.float64(r.results[b]["out"][0, 0]) for b in range(NCORES)]
    loss = np.float32(sum(partials) / (NCORES * S))
    return np.asarray(loss, dtype=np.float32)
